# revision 1
# baseline (speedup 1.0000x reference)
"""MoDA Vision Transformer forward pass on 8 Trainium2 NeuronCores.

Sharding: pure data-parallel over batch (B=8 -> 1 image per core, weights
replicated, no collectives).

v2 design (bf16 compute, fp32 residual):
- All weights pre-tiled + cast to bf16 on HOST into exactly the SBUF layout
  [p, k, c], so every weight DMA is one contiguous chunk per partition
  (128 descriptors per load instead of ~10k strided ones).
- Residual stream hT stays fp32 (f32r) padded to 256 token cols so the
  LayerNorm sum matmuls run at full f32r rate; all other activations are
  bf16 at 208 token cols (bf16 matmuls are full rate at any width).
- im2col + pos_embed/cls/bias folding done on host; patch embed is a
  plain linear directly into the transposed residual layout.
- Attention (GQA kv_heads=1, depth-KV cache): q pieces of (1024,1024,316)
  flattened (g,t) queries; per kpos-block one 1024-wide exp on ACT
  (amortizes the 352-cycle ACTIVATE overhead); denominators via a ones
  column appended to V; softmax normalize with DVE reciprocal.
- PSUM: 3x [128,1024] "big" slots (6 banks) + 2x [1-128,512] "row" slots
  (2 banks) = exactly 8 banks.
- Next layer's weights are prefetched at the top of each layer body on the
  sync HWDGE ring; small strip DMAs ride the scalar HWDGE ring.
"""

import numpy as np
import ml_dtypes

import concourse.bass as bass
import concourse.mybir as mybir
import concourse.tile as tile
from concourse import bacc
from concourse.bass_utils import run_bass_kernel_spmd
from concourse.masks import make_identity

F32 = mybir.dt.float32
F32R = mybir.dt.float32r
BF16 = mybir.dt.bfloat16
I32 = mybir.dt.int32
AF = mybir.ActivationFunctionType
OP = mybir.AluOpType
BFNP = ml_dtypes.bfloat16

B, CIN, IMG, P = 8, 3, 224, 16
DIM, DEPTH, NH, NKV = 768, 12, 12, 1
HD = DIM // NH              # 64
G = NH // NKV               # 12
NPATCH = (IMG // P) ** 2    # 196
T = NPATCH + 1              # 197
TC = 208                    # bf16 activation token cols (197 padded)
TR = 256                    # fp32 residual token cols (f32r full-rate >=256)
KD = DIM // 128             # 6
MLP = 4 * DIM               # 3072
MD = MLP // 128             # 24
NQ = G * T                  # 2364
NQP = 2368                  # padded
NKBLK = (DEPTH * T + 127) // 128  # 19
VW = 130                    # V block width (64 V + 1 ones + 63 zero + pad)
SCALE = HD ** -0.5
EPS = 1e-6
NCLS = 1000
QP_ = [(0, 1024), (1024, 1024), (2048, NQ - 2048)]  # q pieces

# packed per-layer vector columns (fp32): [128, NV]
VO_L1W, VO_L1B, VO_QB, VO_KVB, VO_PB = 0, 6, 12, 18, 19
VO_L2W, VO_L2B, VO_F1B, VO_F2B, NV = 25, 31, 37, 61, 67

_CACHED = {}


def build_module():
    nc = bacc.Bacc("TRN2", target_bir_lowering=False, debug=False, num_devices=8)

    d = {}
    def din(name, shape, dt_):
        d[name] = nc.dram_tensor(name, shape, dt_, kind="ExternalInput")

    din("xpt", (128, KD * TC), BF16)
    din("posbt", (128, KD * TC), F32)
    din("patchw", (128, KD * DIM), BF16)
    din("qw", (DEPTH, 128, KD * DIM), BF16)
    din("kvw", (DEPTH, 128, KD * 2 * HD), BF16)
    din("projw", (DEPTH, 128, KD * DIM), BF16)
    din("fc1w", (DEPTH, 128, KD * MLP), BF16)
    din("fc2w", (DEPTH, 128, MD * DIM), BF16)
    din("vecs", (DEPTH, 128, NV), F32)
    din("normv", (128, 12), F32)
    din("headw", (128, KD * NCLS), BF16)
    din("headb", (NCLS,), F32)
    din("_ones", (128,), F32R)
    din("_zeros", (KD * TR,), F32R)
    out_d = nc.dram_tensor("out", (1, NCLS), F32, kind="ExternalOutput")

    with tile.TileContext(nc) as tc:
        with (
            tc.tile_pool(name="persist", bufs=1) as persist,
            tc.tile_pool(name="wq", bufs=2) as wq,        # q/kv/proj + vecs
            tc.tile_pool(name="wsl", bufs=10) as wsl,     # fc1/fc2 quarter slabs
            tc.tile_pool(name="tmp", bufs=2) as tmp,      # transient sbuf
            tc.tile_pool(name="ps", bufs=1, space="PSUM") as ps,
        ):
            # ---------------- persistent state ----------------
            hT = persist.tile([128, KD, TR], F32R)          # residual ^T (fp32)
            sqT = persist.tile([128, KD, TR], BF16)         # squares scratch
            hnT = persist.tile([128, KD, TC], BF16)         # LN output ^T
            oT = persist.tile([128, KD, TC], BF16)          # attn out ^T
            g1T = persist.tile([128, MD, TC], BF16)         # gelu(fc1) ^T
            KC = persist.tile([128, NQP], BF16)             # K cache ^T (x2 dup)
            VC = persist.tile([128, NKBLK, VW], BF16)       # V cache + ones col
            qpt = persist.tile([128, NQP], BF16)            # Q'^T (g,t) (x2 dup)
            otn = persist.tile([64, NQP], BF16)             # normalized O'^T
            ident = persist.tile([128, 128], F32)
            onec = persist.tile([128, 1], F32R)             # 1.0 col (LN sum lhsT)
            onecb = persist.tile([128, 1], BF16)
            oner = persist.tile([1, 128], F32R)             # 1.0 row (bcast lhsT)
            orow = persist.tile([1, NCLS], F32)

            nc.gpsimd.dma_start(
                hT, d["_zeros"].ap().rearrange("(o c) -> o c", o=1)
                .to_broadcast([128, KD * TR]))
            nc.vector.memset(oT, 0.0)
            nc.vector.memset(VC, 0.0)
            make_identity(nc, ident)
            nc.sync.dma_start(onec, d["_ones"].ap().rearrange("(p o) -> p o", o=1))
            nc.sync.dma_start(oner, d["_ones"].ap().rearrange("(o p) -> o p", o=1))
            nc.vector.memset(onecb, 1.0)
            nc.vector.memset(KC[64:128, :], 0.0)
            nc.vector.memset(qpt[64:128, :], 0.0)
            nc.vector.memset(VC[:, :, HD:HD + 2], 1.0)

            # ---------------- weight loading helpers ----------------
            def load_qkvp(l):
                v = wq.tile([128, NV], F32, name="vecs", tag="vecs")
                nc.sync.dma_start(v, d["vecs"].ap()[l])
                qw_ = wq.tile([128, KD, DIM], BF16, name="qw", tag="qw")
                nc.sync.dma_start(qw_, d["qw"].ap()[l].rearrange("p (k c) -> p k c", k=KD))
                kvw_ = wq.tile([128, KD, 2 * HD], BF16, name="kvw", tag="kvw")
                nc.sync.dma_start(kvw_, d["kvw"].ap()[l].rearrange("p (k c) -> p k c", k=KD))
                ow_ = wq.tile([128, KD, DIM], BF16, name="ow", tag="ow")
                nc.sync.dma_start(ow_, d["projw"].ap()[l].rearrange("p (k c) -> p k c", k=KD))
                return v, qw_, kvw_, ow_

            def load_slabs(l):
                f1r = d["fc1w"].ap()[l].rearrange("p (k c) -> p k c", k=KD)
                f2r = d["fc2w"].ap()[l].rearrange("p (k c) -> p k c", k=MD)
                f1q, f2q = [], []
                for i in range(4):  # fc1 quarter: m-tiles 6i/4.. (768 cols each)
                    s = wsl.tile([128, KD, MLP // 4], BF16, name="f1q", tag="slab")
                    nc.sync.dma_start(s, f1r[:, :, i * (MLP // 4):(i + 1) * (MLP // 4)])
                    f1q.append(s)
                for i in range(4):  # fc2 quarter: k-tiles 6i..6i+5 (full 768 cols)
                    s = wsl.tile([128, KD, DIM], BF16, name="f2q", tag="slab")
                    nc.sync.dma_start(s, f2r[:, i * KD:(i + 1) * KD, :])
                    f2q.append(s)
                return f1q, f2q

            # ---------------- layernorm ----------------
            def ln(dst, wb, wo, bo):
                """LN over d of hT -> dst[128, KD, TC] (bf16).

                wb: [128, NV]-style tile; wo/bo: col offsets of gamma/beta."""
                nc.vector.tensor_tensor(sqT, hT, hT, op=OP.mult)
                ssum = ps.tile([1, 512], F32, name="ssum", tag="row", bufs=2)
                ssq = ps.tile([1, 512], F32, name="ssq", tag="row", bufs=2)
                for k in range(KD):
                    nc.tensor.matmul(ssum[:, 0:TR], onec, hT[:, k, :],
                                     start=(k == 0), stop=(k == KD - 1))
                for k in range(KD):
                    nc.tensor.matmul(ssq[:, 0:TR], onecb, sqT[:, k, :],
                                     start=(k == 0), stop=(k == KD - 1))
                mean = tmp.tile([1, TC], F32, name="mean", tag="mean", bufs=1)
                nc.vector.tensor_scalar(out=mean, in0=ssum[:, 0:TC],
                                        scalar1=1.0 / DIM, scalar2=None, op0=OP.mult)
                m2 = tmp.tile([1, TC], F32, name="m2", tag="m2", bufs=1)
                nc.vector.tensor_tensor(m2, mean, mean, op=OP.mult)
                var = tmp.tile([1, TC], F32, name="var", tag="var", bufs=1)
                nc.vector.scalar_tensor_tensor(
                    out=var, in0=ssq[:, 0:TC], scalar=1.0 / DIM,
                    in1=m2, op0=OP.mult, op1=OP.subtract)
                ve = tmp.tile([1, TC], F32, name="ve", tag="ve", bufs=1)
                nc.vector.tensor_scalar(out=ve, in0=var, scalar1=EPS,
                                        scalar2=None, op0=OP.add)
                sd = tmp.tile([1, TC], I32, name="sd", tag="sd", bufs=1)
                nc.vector.tensor_scalar(out=sd, in0=ve.bitcast(I32), scalar1=1,
                                        scalar2=None, op0=OP.logical_shift_right)
                nc.vector.tensor_scalar(out=sd, in0=sd, scalar1=-1,
                                        scalar2=0x5F3759DF, op0=OP.mult, op1=OP.add)
                y0 = sd.bitcast(F32)
                t_ = tmp.tile([1, TC], F32, name="nrT", tag="nrT", bufs=1)
                nc.vector.tensor_tensor(t_, y0, y0, op=OP.mult)
                nc.vector.tensor_tensor(t_, t_, ve, op=OP.mult)
                nc.vector.tensor_scalar(out=t_, in0=t_, scalar1=-0.5, scalar2=1.5,
                                        op0=OP.mult, op1=OP.add)
                rstd = tmp.tile([1, TC], F32R, name="rstd", tag="rstd", bufs=1)
                nc.vector.tensor_tensor(rstd, y0, t_, op=OP.mult)
                mr = tmp.tile([1, TC], F32R, name="mr", tag="mr", bufs=1)
                nc.vector.tensor_tensor(mr, mean, rstd, op=OP.mult)
                for _ in range(8):
                    dmy = ps.tile([128, 512], F32, name="dmyl", tag="row", bufs=2)
                    nc.tensor.matmul(dmy, KC[:, 0:128], qpt[:, 0:512],
                                     start=True, stop=True)
                rstd_b = ps.tile([128, TC], F32, name="rstd_b", tag="row", bufs=2)
                mr_b = ps.tile([128, TC], F32, name="mr_b", tag="row", bufs=2)
                nc.tensor.matmul(rstd_b, oner, rstd, start=True, stop=True)
                nc.tensor.matmul(mr_b, oner, mr, start=True, stop=True)
                for _ in range(6):
                    dmy = ps.tile([128, 512], F32, name="dmyl", tag="row", bufs=2)
                    nc.tensor.matmul(dmy, KC[:, 0:128], qpt[:, 0:512],
                                     start=True, stop=True)
                for k in range(KD):
                    t1 = tmp.tile([128, TC], F32, name="lnt", tag="lnt", bufs=2)
                    nc.vector.tensor_tensor(t1, hT[:, k, 0:TC], rstd_b, op=OP.mult)
                    nc.vector.tensor_tensor(t1, t1, mr_b, op=OP.subtract)
                    nc.vector.tensor_scalar(
                        out=dst[:, k, :], in0=t1,
                        scalar1=wb[:, wo + k:wo + k + 1],
                        scalar2=wb[:, bo + k:bo + k + 1],
                        op0=OP.mult, op1=OP.add)

            # ---------------- prologue: patch embed ----------------
            xpt = tmp.tile([128, KD, TC], BF16, name="xpt", tag="pt", bufs=2)
            nc.sync.dma_start(xpt, d["xpt"].ap().rearrange("p (k c) -> p k c", k=KD))
            posbt = wsl.tile([128, KD, TC], F32, name="posbt", tag="slab")
            nc.sync.dma_start(posbt, d["posbt"].ap().rearrange("p (k c) -> p k c", k=KD))
            pw_sb = wq.tile([128, KD, DIM], BF16, name="pw_sb", tag="qw")
            nc.sync.dma_start(pw_sb, d["patchw"].ap().rearrange("p (k c) -> p k c", k=KD))
            vecs0 = load_qkvp(0)
            slabs0 = load_slabs(0)

            for m in range(KD):
                pp = ps.tile([128, 1024], F32, name="pp", tag="big", bufs=3)
                for k in range(KD):
                    nc.tensor.matmul(pp[:, 0:TC], pw_sb[:, k, m * 128:(m + 1) * 128],
                                     xpt[:, k, :], start=(k == 0), stop=(k == KD - 1))
                nc.vector.tensor_tensor(hT[:, m, 0:TC], pp[:, 0:TC],
                                        posbt[:, m, :], op=OP.add)

            # ---------------- transformer layers ----------------
            lw = (vecs0, slabs0)
            for l in range(DEPTH):
                (vv, qw_sb, kvw_sb, ow_sb), (f1q, f2q) = lw
                if l + 1 < DEPTH:
                    nxt = (load_qkvp(l + 1), load_slabs(l + 1))

                ln(hnT, vv, VO_L1W, VO_L1B)

                # ---- Q projection -> qpt strips ----
                for m in range(KD):
                    qp = ps.tile([128, 1024], F32, name="qp", tag="big", bufs=3)
                    for k in range(KD):
                        nc.tensor.matmul(qp[:, 0:TC], qw_sb[:, k, m * 128:(m + 1) * 128],
                                         hnT[:, k, :], start=(k == 0), stop=(k == KD - 1))
                    nc.vector.tensor_scalar(
                        out=qpt[0:64, (2 * m) * T:(2 * m) * T + T],
                        in0=qp[0:64, 0:T],
                        scalar1=vv[0:64, VO_QB + m:VO_QB + m + 1], scalar2=None,
                        op0=OP.add)
                    nc.vector.tensor_scalar(
                        out=qpt[0:64, (2 * m + 1) * T:(2 * m + 1) * T + T],
                        in0=qp[64:128, 0:T],
                        scalar1=vv[64:128, VO_QB + m:VO_QB + m + 1], scalar2=None,
                        op0=OP.add)

                # ---- KV projection; append K^T and V to caches ----
                kvp = ps.tile([128, 1024], F32, name="kvp", tag="big", bufs=3)
                for k in range(KD):
                    nc.tensor.matmul(kvp[:, 0:TC], kvw_sb[:, k, :], hnT[:, k, :],
                                     start=(k == 0), stop=(k == KD - 1))
                nc.vector.tensor_scalar(
                    out=KC[0:64, l * T:l * T + T], in0=kvp[0:64, 0:T],
                    scalar1=vv[0:64, VO_KVB:VO_KVB + 1], scalar2=None, op0=OP.add)
                vsb = tmp.tile([128, TC], F32, name="vsb", tag="vsb", bufs=1)
                nc.vector.tensor_scalar(
                    out=vsb[64:128, :], in0=kvp[64:128, 0:TC],
                    scalar1=vv[64:128, VO_KVB:VO_KVB + 1], scalar2=None, op0=OP.add)
                for tc_i, tsz in ((0, 128), (1, 69)):
                    vtp = ps.tile([128, 512], F32, name="vtp", tag="row", bufs=2)
                    nc.tensor.matmul(vtp[0:tsz, 0:HD],
                                     vsb[64:128, tc_i * 128:tc_i * 128 + tsz],
                                     ident[64:128, 64:64 + HD], is_transpose=True,
                                     start=True, stop=True)
                    vts = tmp.tile([128, HD], BF16, name="vts", tag="vts", bufs=2)
                    nc.vector.tensor_copy(vts[0:tsz], vtp[0:tsz, 0:HD])
                    t0 = 0
                    while t0 < tsz:
                        kpos = l * T + tc_i * 128 + t0
                        blk, off = kpos // 128, kpos % 128
                        cnt = min(tsz - t0, 128 - off)
                        nc.scalar.dma_start(
                            VC[off:off + cnt, blk, 0:HD],
                            vts[t0:t0 + cnt, :])
                        t0 += cnt

                # ---- attention ----
                Lk = (l + 1) * T
                nkt = (Lk + 127) // 128
                for qoff, qsz in QP_:
                    nh = (qsz + 511) // 512
                    ot = ps.tile([128, 1024], F32, name="ot", tag="big", bufs=3)
                    for c in range(nkt):
                        ksz = min(128, Lk - c * 128)
                        st = ps.tile([128, 1024], F32, name="st", tag="big", bufs=3)
                        for h in range(nh):
                            cw = min(512, qsz - h * 512)
                            nc.tensor.matmul(
                                st[0:ksz, h * 512:h * 512 + cw],
                                KC[:, c * 128:c * 128 + ksz],
                                qpt[:, qoff + h * 512:qoff + h * 512 + cw],
                                start=True, stop=True)
                        dmy = ps.tile([128, 512], F32, name="dmy", tag="row",
                                      bufs=2)
                        nc.tensor.matmul(dmy, KC[:, 0:128], qpt[:, 0:512],
                                         start=True, stop=True)
                        pt = tmp.tile([128, 1024], BF16, name="pt", tag="pt", bufs=2)
                        nc.scalar.activation(pt[0:ksz, 0:qsz], st[0:ksz, 0:qsz],
                                             AF.Exp, scale=SCALE)
                        for h in range(nh):
                            cw = min(512, qsz - h * 512)
                            nc.tensor.matmul(
                                ot[:, h * 512:h * 512 + cw],
                                VC[0:ksz, c, 0:128],
                                pt[0:ksz, h * 512:h * 512 + cw],
                                start=(c == 0), stop=(c == nkt - 1))
                    # evacuate ot fast (frees the PSUM slot for the next
                    # piece), then normalize off the critical path
                    ots = tmp.tile([65, 1024], F32, name="ots", tag="ots", bufs=2)
                    nc.vector.tensor_copy(ots[:, 0:qsz], ot[0:65, 0:qsz])
                    for h in range(nh):
                        cw = min(512, qsz - h * 512)
                        denr = tmp.tile([1, 512], F32R, name="denr", tag="rec", bufs=2)
                        nc.vector.tensor_copy(denr[:, 0:cw],
                                              ots[64:65, h * 512:h * 512 + cw])
                        rbp = ps.tile([64, 512], F32, name="rbp", tag="row", bufs=2)
                        nc.tensor.matmul(rbp[:, 0:cw], oner[:, 0:64], denr[:, 0:cw],
                                         start=True, stop=True)
                        rb = tmp.tile([64, 512], F32, name="rb", tag="rb", bufs=2)
                        nc.vector.reciprocal_approx_fast(rb[:, 0:cw], rbp[:, 0:cw])
                        nc.vector.tensor_tensor(
                            otn[:, qoff + h * 512:qoff + h * 512 + cw],
                            ots[0:64, h * 512:h * 512 + cw], rb[:, 0:cw],
                            op=OP.mult)

                # ---- reshape O'T (g,t) -> oT [d, t] ----
                for g in range(G):
                    j, half = g // 2, g % 2
                    nc.vector.tensor_copy(oT[64 * half:64 * half + 64, j, 0:T],
                                          otn[:, g * T:g * T + T])

                # ---- output projection + residual ----
                for _ in range(6):
                    dmy = ps.tile([128, 512], F32, name="dmyp", tag="row", bufs=2)
                    nc.tensor.matmul(dmy, KC[:, 0:128], qpt[:, 0:512],
                                     start=True, stop=True)
                for m in range(KD):
                    op_ = ps.tile([128, 1024], F32, name="prp", tag="big", bufs=3)
                    for k in range(KD):
                        nc.tensor.matmul(op_[:, 0:TC], ow_sb[:, k, m * 128:(m + 1) * 128],
                                         oT[:, k, :], start=(k == 0), stop=(k == KD - 1))
                    nc.vector.scalar_tensor_tensor(
                        out=hT[:, m, 0:T], in0=op_[:, 0:T],
                        scalar=vv[:, VO_PB + m:VO_PB + m + 1],
                        in1=hT[:, m, 0:T], op0=OP.add, op1=OP.add)

                # ---- MLP ----
                ln(hnT, vv, VO_L2W, VO_L2B)
                for m in range(MD):
                    f1s = f1q[m // 6]
                    mi = m % 6
                    fp = ps.tile([128, 1024], F32, name="fp", tag="big", bufs=3)
                    for k in range(KD):
                        nc.tensor.matmul(fp[:, 0:TC], f1s[:, k, mi * 128:(mi + 1) * 128],
                                         hnT[:, k, :], start=(k == 0), stop=(k == KD - 1))
                    nc.scalar.activation(g1T[:, m, :], fp[:, 0:TC], AF.Gelu,
                                         bias=vv[:, VO_F1B + m:VO_F1B + m + 1])
                for m in range(KD):
                    f2p = ps.tile([128, 1024], F32, name="f2p", tag="big", bufs=3)
                    for k in range(MD):
                        f2s = f2q[k // 6]
                        nc.tensor.matmul(f2p[:, 0:TC],
                                         f2s[:, k % 6, m * 128:(m + 1) * 128],
                                         g1T[:, k, :], start=(k == 0), stop=(k == MD - 1))
                    nc.vector.scalar_tensor_tensor(
                        out=hT[:, m, 0:T], in0=f2p[:, 0:T],
                        scalar=vv[:, VO_F2B + m:VO_F2B + m + 1],
                        in1=hT[:, m, 0:T], op0=OP.add, op1=OP.add)

                if l + 1 < DEPTH:
                    lw = nxt

            # ---------------- final LN + head ----------------
            nv = persist.tile([128, 12], F32)
            nc.sync.dma_start(nv, d["normv"].ap())
            nc.sync.dma_start(orow, d["headb"].ap().rearrange("(o c) -> o c", o=1))
            ln(hnT, nv, 0, 6)
            hwr = d["headw"].ap().rearrange("p (k c) -> p k c", k=KD)
            for n in range(2):
                hw_c = wsl.tile([128, KD, 500], BF16, name="hw_c", tag="slab")
                nc.sync.dma_start(hw_c, hwr[:, :, n * 500:(n + 1) * 500])
                hp = ps.tile([1, 512], F32, name="hp", tag="row", bufs=2)
                for k in range(KD):
                    nc.tensor.matmul(hp[:, 0:500], hnT[:, k, 0:1], hw_c[:, k, :],
                                     start=(k == 0), stop=(k == KD - 1))
                nc.vector.tensor_tensor(orow[:, n * 500:(n + 1) * 500], hp[:, 0:500],
                                        orow[:, n * 500:(n + 1) * 500], op=OP.add)
            nc.sync.dma_start(out_d.ap(), orow)

    nc.compile()
    return nc


def _tile_w(w):
    """(K*128, C) fp32 -> (128, K*C) bf16 tiled: out[p, k*C+c] = w[k*128+p, c]."""
    k = w.shape[0] // 128
    c = w.shape[1]
    return np.ascontiguousarray(
        w.reshape(k, 128, c).transpose(1, 0, 2).reshape(128, k * c).astype(BFNP))


def _vcol(v):
    """(K*128,) -> (128, K): out[p, k] = v[k*128+p]."""
    k = v.shape[0] // 128
    return v.reshape(k, 128).T


def make_in_maps(inputs):
    f = {n: np.asarray(inputs[n], dtype=np.float32) for n in inputs}

    shared = {}
    shared["patchw"] = _tile_w(f["patch_w"])
    shared["qw"] = np.stack([_tile_w(f["q_w"][l]) for l in range(DEPTH)])
    shared["kvw"] = np.stack([_tile_w(f["kv_w"][l]) for l in range(DEPTH)])
    shared["projw"] = np.stack([_tile_w(f["proj_w"][l]) for l in range(DEPTH)])
    shared["fc1w"] = np.stack([_tile_w(f["fc1_w"][l]) for l in range(DEPTH)])
    shared["fc2w"] = np.stack([_tile_w(f["fc2_w"][l]) for l in range(DEPTH)])
    shared["headw"] = _tile_w(f["head_w"])
    shared["headb"] = f["head_b"]

    vecs = np.zeros((DEPTH, 128, NV), np.float32)
    for l in range(DEPTH):
        vecs[l, :, VO_L1W:VO_L1W + 6] = _vcol(f["ln1_w"][l])
        vecs[l, :, VO_L1B:VO_L1B + 6] = _vcol(f["ln1_b"][l])
        vecs[l, :, VO_QB:VO_QB + 6] = _vcol(f["q_b"][l])
        vecs[l, :, VO_KVB] = f["kv_b"][l]
        vecs[l, :, VO_PB:VO_PB + 6] = _vcol(f["proj_b"][l])
        vecs[l, :, VO_L2W:VO_L2W + 6] = _vcol(f["ln2_w"][l])
        vecs[l, :, VO_L2B:VO_L2B + 6] = _vcol(f["ln2_b"][l])
        vecs[l, :, VO_F1B:VO_F1B + 24] = _vcol(f["fc1_b"][l])
        vecs[l, :, VO_F2B:VO_F2B + 6] = _vcol(f["fc2_b"][l])
    shared["vecs"] = np.ascontiguousarray(vecs)

    normv = np.zeros((128, 12), np.float32)
    normv[:, 0:6] = _vcol(f["norm_w"])
    normv[:, 6:12] = _vcol(f["norm_b"])
    shared["normv"] = normv

    # pos_embed + patch_b / cls folding, transposed token layout
    posb = np.zeros((DIM, TC), np.float32)
    posb[:, 0] = f["cls_token"][0, 0] + f["pos_embed"][0, 0]
    posb[:, 1:T] = (f["pos_embed"][0, 1:T] + f["patch_b"][None, :]).T
    shared["posbt"] = np.ascontiguousarray(
        posb.reshape(KD, 128, TC).transpose(1, 0, 2).reshape(128, KD * TC))

    shared["_ones"] = np.ones((128,), np.float32)
    shared["_zeros"] = np.zeros((KD * TR,), np.float32)

    # per-core im2col (transposed): xpt[(c,a,b), 1 + i*14 + j]
    HG = IMG // P
    x = np.asarray(inputs["x"], dtype=np.float32)
    maps = []
    for b in range(B):
        xp = x[b].reshape(CIN, HG, P, HG, P).transpose(0, 2, 4, 1, 3)
        xp = xp.reshape(DIM, NPATCH)
        xt = np.zeros((DIM, TC), np.float32)
        xt[:, 1:T] = xp
        xt = xt.reshape(KD, 128, TC).transpose(1, 0, 2).reshape(128, KD * TC)
        maps.append(dict(shared, xpt=np.ascontiguousarray(xt.astype(BFNP))))
    return maps


def kernel(**inputs):
    if "nc" not in _CACHED:
        _CACHED["nc"] = build_module()
    nc = _CACHED["nc"]
    res = run_bass_kernel_spmd(nc, make_in_maps(inputs), core_ids=list(range(B)))
    return np.concatenate([res.results[b]["out"] for b in range(B)], axis=0)

